# revision 32
# baseline (speedup 1.0000x reference)
"""Attention with full pair dedup: K and V each computed for own half
only and exchanged via 2-rank AllGathers (v4.5).

Per-core matmul streaming is clock-bound (~0.42-0.51 ns/col depending on
the brokered silicon) with zero per-matmul bubble, so the wins over the
v3 baseline are fewer streamed columns and hiding the collective chain:
  - K pair-dedup (-36864 cols ~ -17us): KT computed for the own half
    only and exchanged like V, one single gather per tensor (each CC op
    pays ~11.5us serial notice latency, so never split a gather).
  - Input DMAs in strict need-order over 3 queues (early DMA bandwidth
    ramps slowly; wk+slab0 land first so KT starts ~13us).
  - K readback pipelined in kc-pair chunks over sync/scalar/gpsimd in
    consumption order: scores start ~1.5us after the gather lands.
  - kc-major scores order: the first 24 groups read only the first
    rank-block of kt, giving the gathered half ~28us of extra slack
    against the run-to-run CC barrier jitter (10-90us rendezvous).
  - Output staged fp16 (halves out-DMA + DVE normalize; +2e-4 rel err
    against a 2e-2 gate), per-chunk normalize off the critical tail.

Slot discipline: gather output is rank-ordered == physical key order
(rank0 = first half) on both cores, so kt/vp/et chunk indexing is
rank-agnostic everywhere."""

import numpy as np

import concourse.bass as bass
import concourse.mybir as mybir
import concourse.tile as tile
from concourse import bacc
from concourse.bass_utils import run_bass_kernel_spmd

N_CORES = 8
B, N, D, OUT = 4, 2048, 768, 768
NQ = N // 2
P = 128
DC = D // P
OC = OUT // P
KC = N // P
HKC = KC // 2  # k-chunks per half
F32 = mybir.dt.float32
FP16 = mybir.dt.float16
PAIRS = [[0, 1], [2, 3], [4, 5], [6, 7]]

Q_BLOCKS = [(0, 384), (384, 384), (768, 256)]
N_WARMUP = 13


def build_attention_nc():
    nc = bacc.Bacc("TRN2", target_bir_lowering=False, debug=False)
    xq = nc.dram_tensor("xq", [D, NQ], FP16, kind="ExternalInput")
    w = nc.dram_tensor("w", [3, D, OUT], FP16, kind="ExternalInput")
    out = nc.dram_tensor("out", [NQ, OUT], FP16, kind="ExternalOutput")

    with tile.TileContext(nc) as tc:
        with (
            tc.tile_pool(name="persist", bufs=1) as persist,
            tc.tile_pool(name="dpool", bufs=1, space="DRAM") as dpool,
        ):
            qt = persist.tile([P, OC, NQ], FP16)  # QT[o,q] (local only)
            kt = persist.tile([P, OC, N], FP16)  # KT[o,k] physical order
            vp = persist.tile([P, KC, OUT + 2], FP16)  # V' physical order

            kpb_in = dpool.tile([P, OC, NQ], FP16, name="kpb_in")
            kpb_out = dpool.tile([2, P, OC, NQ], FP16, name="kpb_out")
            vpb_in = dpool.tile([P, HKC, OUT + 2], FP16)
            vpb_out = dpool.tile([2, P, HKC, OUT + 2], FP16)

            ones_sc = persist.tile([P, 1], F32, name="ones_sc")
            nc.vector.memset(ones_sc, 1.0)
            zero_sc = persist.tile([P, 1], F32, name="zero_sc")
            nc.vector.memset(zero_sc, 0.0)

            with (
                tc.tile_pool(name="slabs", bufs=2) as slabs,
                tc.tile_pool(name="psa", bufs=6, space="PSUM") as psa,
                tc.tile_pool(name="wpool", bufs=1) as wpool,
                tc.tile_pool(name="stage", bufs=4) as stage,
            ):
                wk_sb = wpool.tile([P, DC, OUT], FP16)
                wq_sb = wpool.tile([P, DC, OUT], FP16)
                wv_sb = wpool.tile([P, DC, OUT], FP16)

                # HAM warmup while the first DMAs fly
                warm = wpool.tile([P, 512], FP16, name="warm")
                nc.gpsimd.memset(warm, 1.0)
                wps = psa.tile([P, 512], F32, name="wps", bufs=1)
                for i in range(N_WARMUP):
                    nc.tensor.matmul(
                        wps,
                        warm[:, 0:P],
                        warm,
                        start=(i == 0),
                        stop=(i == N_WARMUP - 1),
                    )

                # DMAs in strict need-order (early DMA bandwidth ramps
                # slowly, so first-needed data must have nothing queued
                # ahead): wk+slab0 interleaved on sync/gpsimd, then
                # slab1, wv (V'), wq (QT).
                qslab_tiles = []
                for s in range(2):
                    qslab = slabs.tile(
                        [P, DC, 512], FP16, tag="slab", name=f"qslab{s}"
                    )
                    qslab_tiles.append(qslab)
                qs = [nc.sync, nc.gpsimd, nc.scalar]
                for dc in range(DC):
                    qs[(2 * dc) % 3].dma_start(
                        out=wk_sb[:, dc, :], in_=w[1][dc * P : (dc + 1) * P, :]
                    )
                    qs[(2 * dc + 1) % 3].dma_start(
                        out=qslab_tiles[0][:, dc, :],
                        in_=xq[dc * P : (dc + 1) * P, 0:512],
                    )
                # slab1/wv/wq stay off scalar: the kst staging DMAs get
                # the scalar queue to themselves so the K gather triggers
                # right after the last KT group
                for dc in range(DC):
                    qs[dc % 2].dma_start(
                        out=qslab_tiles[1][:, dc, :],
                        in_=xq[dc * P : (dc + 1) * P, 512:1024],
                    )
                for dc in range(DC):
                    qs[(dc + 1) % 2].dma_start(
                        out=wv_sb[:, dc, :], in_=w[2][dc * P : (dc + 1) * P, :]
                    )
                for dc in range(DC):
                    qs[dc % 2].dma_start(
                        out=wq_sb[:, dc, :], in_=w[0][dc * P : (dc + 1) * P, :]
                    )

                # ---- A1: KT own half -> single pair gather ----
                # the last group's staging is split in 256-col halves on
                # parallel queues so the gather triggers ~1us earlier
                for s in range(2):
                    slab = qslab_tiles[s]
                    for oc in range(OC):
                        last = s == 1 and oc == OC - 1
                        kst = stage.tile([P, 512], FP16, tag="kst", bufs=6)
                        if last:
                            # two 256-key accumulation runs in separate
                            # psum tiles: the first half's copy+DMA
                            # overlaps the second half's matmuls
                            for hf, eng in ((0, nc.scalar), (1, nc.sync)):
                                lo, hi = hf * 256, (hf + 1) * 256
                                ph = psa.tile([P, 256], F32, tag="psah", bufs=1)
                                for dc in range(DC):
                                    nc.tensor.matmul(
                                        ph,
                                        wk_sb[:, dc, oc * P : (oc + 1) * P],
                                        slab[:, dc, lo:hi],
                                        start=(dc == 0),
                                        stop=(dc == DC - 1),
                                    )
                                nc.vector.tensor_copy(kst[:, lo:hi], ph)
                                eng.dma_start(
                                    out=kpb_in[
                                        :, oc, s * 512 + lo : s * 512 + hi
                                    ],
                                    in_=kst[:, lo:hi],
                                )
                        else:
                            ps = psa.tile([P, 512], F32, tag="psa")
                            for dc in range(DC):
                                nc.tensor.matmul(
                                    ps,
                                    wk_sb[:, dc, oc * P : (oc + 1) * P],
                                    slab[:, dc, :],
                                    start=(dc == 0),
                                    stop=(dc == DC - 1),
                                )
                            nc.vector.tensor_copy(kst, ps)
                            eng = nc.scalar if oc % 2 == 0 else nc.sync
                            eng.dma_start(
                                out=kpb_in[:, oc, s * 512 : (s + 1) * 512],
                                in_=kst,
                            )
                nc.gpsimd.collective_compute(
                    "AllGather",
                    mybir.AluOpType.bypass,
                    replica_groups=PAIRS,
                    ins=[kpb_in.opt()],
                    outs=[kpb_out.opt()],
                )
                # readback in geometrically growing chunks cycling over
                # the 3 DMA queues, in scores consumption order: scores
                # eat one kc (196KB) per ~2.6us, so a tiny first chunk
                # lets the first group start ~1.8us after the gather
                # lands instead of waiting a fat block. scalar is idle
                # until the exps start.
                kb_chunks = [
                    (0, 0, 128, nc.sync),
                    (0, 128, 256, nc.scalar),
                    (0, 256, 512, nc.gpsimd),
                    (0, 512, 768, nc.sync),
                    (0, 768, 1024, nc.scalar),
                    (1, 0, 256, nc.gpsimd),
                    (1, 256, 512, nc.sync),
                    (1, 512, 768, nc.scalar),
                    (1, 768, 1024, nc.gpsimd),
                ]
                for h, lo, hi, eng in kb_chunks:
                    eng.dma_start(
                        out=kt[:, :, h * NQ + lo : h * NQ + hi],
                        in_=kpb_out[h][:, :, lo:hi],
                    )

                # ---- A2: V' half -> gather ----
                for s in range(2):
                    slab = qslab_tiles[s]
                    for j in range(4):
                        kc = s * 4 + j
                        ps1 = psa.tile([P, 512], F32, tag="psa")
                        ps2 = psa.tile([P, 512], F32, tag="psa")
                        for dc in range(DC):
                            nc.tensor.matmul(
                                ps1[:, 0:384],
                                slab[:, dc, j * P : (j + 1) * P],
                                wv_sb[:, dc, 0:384],
                                start=(dc == 0),
                                stop=(dc == DC - 1),
                            )
                            nc.tensor.matmul(
                                ps2[:, 0:384],
                                slab[:, dc, j * P : (j + 1) * P],
                                wv_sb[:, dc, 384:OUT],
                                start=(dc == 0),
                                stop=(dc == DC - 1),
                            )
                        # copies split DVE/ACT: DVE was locally saturated
                        # when it carried all four (measured micro-stalls)
                        vst = stage.tile([P, OUT + 2], FP16, tag="vst", bufs=9)
                        nc.vector.tensor_copy(vst[:, 0:384], ps1[:, 0:384])
                        nc.scalar.copy(vst[:, 384:OUT], ps2[:, 0:384])
                        nc.scalar.copy(vst[:, OUT : OUT + 1], ones_sc)
                        nc.scalar.copy(vst[:, OUT + 1 : OUT + 2], zero_sc)
                        nc.gpsimd.dma_start(out=vpb_in[:, kc, :], in_=vst)
                nc.gpsimd.collective_compute(
                    "AllGather",
                    mybir.AluOpType.bypass,
                    replica_groups=PAIRS,
                    ins=[vpb_in.opt()],
                    outs=[vpb_out.opt()],
                )
                # V readback split across sync/gpsimd only — a scalar
                # chunk would block the in-order scalar queue (gated on
                # the V collective) ahead of the exps and stall scores
                for h in range(2):
                    for kp in range(2):
                        eng = nc.sync if (h * 2 + kp) % 2 == 0 else nc.gpsimd
                        lo, hi = kp * 4, (kp + 1) * 4
                        eng.dma_start(
                            out=vp[:, h * HKC + lo : h * HKC + hi, :],
                            in_=vpb_out[h][:, lo:hi, :],
                        )

                # ---- A3: QT half (local) ----
                for s in range(2):
                    slab = qslab_tiles[s]
                    for oc in range(OC):
                        ps = psa.tile([P, 512], F32, tag="psa")
                        for dc in range(DC):
                            nc.tensor.matmul(
                                ps,
                                wq_sb[:, dc, oc * P : (oc + 1) * P],
                                slab[:, dc, :],
                                start=(dc == 0),
                                stop=(dc == DC - 1),
                            )
                        nc.vector.tensor_copy(
                            qt[:, oc, s * 512 : (s + 1) * 512], ps
                        )

            # ---- phase B: all scoresT runs, then all out runs ----
            with (
                tc.tile_pool(name="expp", bufs=50) as expp,
                tc.tile_pool(name="obp", bufs=4) as obp,
                tc.tile_pool(name="smallp", bufs=4) as smallp,
                tc.tile_pool(name="ps_sc", bufs=2, space="PSUM") as ps_sc,
                tc.tile_pool(name="ps_out", bufs=3, space="PSUM") as ps_out,
            ):
                # kc-major order: the first 24 groups touch only kt's
                # first rank-block (keys 0-1023), giving the gathered
                # second half ~28us of extra slack to arrive
                ets = {}
                for kc in range(KC):
                    for bi, (q0, qb) in enumerate(Q_BLOCKS):
                        st = ps_sc.tile([P, 384], F32, tag="sc")
                        for oc in range(OC):
                            nc.tensor.matmul(
                                st[:, 0:qb],
                                kt[:, oc, kc * P : (kc + 1) * P],
                                qt[:, oc, q0 : q0 + qb],
                                start=(oc == 0),
                                stop=(oc == OC - 1),
                            )
                        et = expp.tile(
                            [P, 384], FP16, tag="exp", name=f"et{bi}_{kc}"
                        )
                        nc.scalar.activation(
                            et[:, 0:qb],
                            st[:, 0:qb],
                            mybir.ActivationFunctionType.Exp,
                            scale=0.125,
                        )
                        ets[(bi, kc)] = et
                # per chunk: the 512:770 run (holding the denominator
                # column) goes FIRST so recip + the 512:768 normalize and
                # its out-DMA half overlap the 0:512 run — only the last
                # mul + half-DMA remain serial after the final matmul
                for bi, (q0, qb) in enumerate(Q_BLOCKS):
                    nqc = qb // P
                    for j in range(nqc):
                        # two separate PSUM tiles so Tile's (tile-granular)
                        # dep tracking lets recip + the 512:768 normalize
                        # + its DMA half truly overlap the 0:512 run
                        op2 = ps_out.tile(
                            [P, OUT + 2 - 512],
                            F32,
                            tag="out2",
                            bufs=3,
                            name=f"o2_{bi}_{j}",
                        )
                        op1 = ps_out.tile(
                            [P, 512], F32, tag="out", name=f"o1_{bi}_{j}"
                        )
                        for kc in range(KC):
                            nc.tensor.matmul(
                                op2,
                                ets[(bi, kc)][:, j * P : (j + 1) * P],
                                vp[:, kc, 512 : OUT + 2],
                                start=(kc == 0),
                                stop=(kc == KC - 1),
                            )
                        for kc in range(KC):
                            nc.tensor.matmul(
                                op1,
                                ets[(bi, kc)][:, j * P : (j + 1) * P],
                                vp[:, kc, 0:512],
                                start=(kc == 0),
                                stop=(kc == KC - 1),
                            )
                        recip = smallp.tile([P, 1], F32, tag="recip")
                        nc.vector.reciprocal(
                            recip, op2[:, OUT - 512 : OUT - 511]
                        )
                        ob = obp.tile([P, OUT], FP16, tag="ob")
                        nc.vector.tensor_scalar_mul(
                            ob[:, 512:OUT], op2[:, 0 : OUT - 512], recip
                        )
                        nc.gpsimd.dma_start(
                            out=out[q0 + j * P : q0 + (j + 1) * P, 512:OUT],
                            in_=ob[:, 512:OUT],
                        )
                        nc.vector.tensor_scalar_mul(ob[:, 0:512], op1, recip)
                        nc.sync.dma_start(
                            out=out[q0 + j * P : q0 + (j + 1) * P, 0:512],
                            in_=ob[:, 0:512],
                        )
    nc.finalize()
    return nc


_NC_CACHE = None


def _get_nc():
    global _NC_CACHE
    if _NC_CACHE is None:
        _NC_CACHE = build_attention_nc()
    return _NC_CACHE


def make_in_maps(x, kernel):
    x = np.asarray(x, dtype=np.float32)
    w = np.ascontiguousarray(
        np.asarray(kernel, dtype=np.float32).astype(np.float16)
    )
    in_maps = []
    for core in range(N_CORES):
        b, half = core // 2, core % 2
        xt16 = x[b].T.astype(np.float16)
        xq = np.ascontiguousarray(xt16[:, half * NQ : (half + 1) * NQ])
        in_maps.append({"xq": xq, "w": w})
    return in_maps


def assemble_output(results):
    out = np.empty((B, N, OUT), dtype=np.float32)
    for core in range(N_CORES):
        b, half = core // 2, core % 2
        out[b, half * NQ : (half + 1) * NQ, :] = results[core]["out"].astype(
            np.float32
        )
    return out


def run_on_hw(x, kernel, trace=False):
    nc = _get_nc()
    res = run_bass_kernel_spmd(
        nc, make_in_maps(x, kernel), list(range(N_CORES)), trace=trace
    )
    return assemble_output(res.results), res


def kernel(x, kernel):
    out, _ = run_on_hw(x, kernel, trace=False)
    return out
